# revision 37
# baseline (speedup 1.0000x reference)
"""COIL-style sparse-attention scoring kernel for Trainium2 (8 NeuronCores).

Reference computation:
    scores[q,i,d,j] = <query_tok_embs[q,i], doc_tok_embs[d,j]>         (K=32)
    masked = where(query_ids[q,i]==doc_ids[d,j], scores, 0)
    tok    = masked.max(axis=j)
    tok_scores[q,d] = sum_i w[q,i] * tok[q,i,d]    (w drops CLS + SEP)
    out = tok_scores + query_cls_emb @ doc_cls_emb.T

Data-parallel over the 64 queries (8 per core, 2 row-blocks of 128 rows =
4 queries x 32 tokens); doc side replicated.

Device algorithm (fp8 DoubleRow + sum-decode + candidate elision):

  * Per-core candidate filter (host, exact): a doc position whose token id
    matches none of the core's 8 queries' token ids can never produce a
    full id match, so its masked contribution is exactly 0 -- it is pruned
    from that core's column set (inverted-index-style work elision; ~5%
    of positions survive for random ids).

  * The cartesian score+match matmul runs as fp8(e4m3) DoubleRow with
    K=160 packed as 80 partition-pairs:
      pairs 0..31:  query (q_hi[e], q_lo[e]) x  doc (d_hi[e], d_hi[e])
      pairs 32..63: query (q_hi[e], 0)       x  doc (d_lo[e], 0)
      pairs 64..78: query 128*onehot pairs   x  doc onehot pairs
      pair  79:     query (-160, -160)       x  doc (2, 2)
    giving  aug = score + 128*(#matching base-6 id digits) - 640  in PSUM.
    A full 5-digit id match makes aug = score; otherwise aug <= score-128.
  * relu(aug) == the where-masked score.  The per-(token,doc) max over doc
    positions j is replaced by a SUM of relu(aug) over j, exact because doc
    positions are permuted host-side (doc data only) into duplicate-rank
    groups: within a group no id repeats inside a doc, so each (token,doc)
    row has at most one match per group.  tok = max over the few groups.
  * relu decode: fp32-PSUM -> fp8-SBUF tensor_scalar/activation split
    across Vector AND Scalar engines (the only PSUM-readers), contiguous
    writes in raw PSUM order.
  * j-sum on TensorE: per phase-pair, one fp8 DoubleRow matmul with
    identity-pair weights accumulates r[.,2p]+r[.,2p+1] over all (row,doc)
    into a [128, 2*128] PSUM tile; the strided rhs AP does the reorder.
  * Group-max on VectorE ([128, 256] tiles), then CLS (bf16 K=768) and the
    weighted token sum (K=128) accumulate into one [8, 128] PSUM tile.
"""

import numpy as np
import ml_dtypes
from contextlib import ExitStack

import concourse.bass as bass
import concourse.bacc as bacc
import concourse.mybir as mybir
import concourse.tile as tile
from concourse.bass_utils import run_bass_kernel_spmd

F32 = mybir.dt.float32
F16 = mybir.dt.float16
BF16 = mybir.dt.bfloat16
FP8 = mybir.dt.float8e4
E4 = ml_dtypes.float8_e4m3

# problem shape (hardcoded per contract)
BQ, LQ, BD, LD, TOK_D, CLS_D = 64, 32, 128, 192, 32, 768
NCORES = 8
QPC = BQ // NCORES          # 8 queries per core
NBLK = 2                    # two row-blocks of 128 = 4 queries x 32 tokens
ROWS = 128
DIG = 6                     # digit base; 6^5 = 7776 > 5000 vocab
NDIG = 5
KD = NDIG * DIG             # 30 one-hot dims
KP = 2 * TOK_D + KD // 2 + 1    # 80 partition-pairs (K=160)
C = 128.0                   # per-digit match bonus
OFF = NDIG * C              # 640 full-match offset
BIAS = -OFF / 4             # -160: fp8-exact (|x|<=240), x2 via the pair
                            # and x2 via the doc-side bias value of 2.0
# group sizes: multiples of 4 so BD*gr is 512-col bank aligned
def _round_g(n):
    return max(4, int(np.ceil(n / 4)) * 4)


def _chunks(ndp):
    """rhs DMA chunk boundaries (cols); small leading chunks, 1024-aligned."""
    cb = [0, 1024]
    while cb[-1] + 6144 < ndp:
        cb.append(cb[-1] + 6144)
    cb = [c for c in cb if c < ndp]
    cb.append(ndp)
    return cb


def build_nc(gsizes, debug_taps=False):
    """gsizes: tuple of per-doc group slot counts (region 0 %4, rest even)."""
    assert gsizes[0] % 4 == 0 and all(g % 2 == 0 for g in gsizes[1:])
    ngrp = len(gsizes)
    ndp = BD * sum(gsizes)              # doc-position columns per block
    reg_off = np.cumsum([0] + [BD * g for g in gsizes]).tolist()
    cb = _chunks(ndp)

    nc = bacc.Bacc(
        "TRN2",
        target_bir_lowering=False,
        debug=False,
        num_devices=NCORES,
    )

    # qlhsT[b]: [KP, 2, ROWS] fp8 pair-major weights per block
    qlhsT_d = nc.dram_tensor("qlhsT", [NBLK, KP, 2 * ROWS], FP8, kind="ExternalInput")
    # rhs chunk-major: chunk c holds [KP, w_c * 2] fp8 (pair slot innermost)
    rhs_ds = [
        nc.dram_tensor(f"rhs{i}", [KP, (cb[i + 1] - cb[i]) * 2], FP8,
                       kind="ExternalInput")
        for i in range(len(cb) - 1)
    ]
    sel_d = nc.dram_tensor("sel", [ROWS, NBLK * QPC], BF16, kind="ExternalInput")
    qclsT_d = nc.dram_tensor("qclsT", [128, 6 * QPC], BF16, kind="ExternalInput")
    dclsT_d = nc.dram_tensor("dclsT", [128, 6 * BD], BF16, kind="ExternalInput")
    idp_d = nc.dram_tensor("idp", [128, 2 * 128], FP8, kind="ExternalInput")
    out_d = nc.dram_tensor("out", [QPC, BD], F32, kind="ExternalOutput")
    if debug_taps:
        r0dbg_d = nc.dram_tensor("r0dbg", [128, NBLK * BD * gsizes[0]], FP8, kind="ExternalOutput")
        tokdbg_d = nc.dram_tensor("tokdbg", [128, NBLK * BD], BF16, kind="ExternalOutput")
        tdbg_d = nc.dram_tensor("tdbg", [128, ngrp * NBLK * BD], F32, kind="ExternalOutput")

    with tile.TileContext(nc) as tc, ExitStack() as ctx:
        const = ctx.enter_context(tc.tile_pool(name="const", bufs=1))
        psum = ctx.enter_context(tc.tile_pool(name="psum", bufs=3, space="PSUM"))
        tpsum = ctx.enter_context(tc.tile_pool(name="tpsum", bufs=1, space="PSUM"))
        opsum = ctx.enter_context(tc.tile_pool(name="opsum", bufs=1, space="PSUM"))
        work = ctx.enter_context(tc.tile_pool(name="work", bufs=1))

        # --- SBUF tiles ---
        rhs_t = const.tile([KP, 2 * ndp], FP8, tag="rhs")       # [p, (col, pair)]
        qlhsT_t = const.tile([KP, NBLK * 2 * ROWS], FP8, tag="qlhsT")
        sel_t = const.tile([ROWS, NBLK * QPC], BF16, tag="sel")
        qclsT_t = const.tile([128, 6 * QPC], BF16, tag="qclsT")
        dclsT_t = const.tile([128, 6 * BD], BF16, tag="dclsT")
        idp_t = const.tile([128, 2 * 128], FP8, tag="idp")
        # relu'd decode, raw PSUM column order:
        #   r0[p, (block, group, doc-in-group, phase)]   (phase innermost)
        #   r1+[p, (block, doc, slot)]                   (slot innermost)
        r_ts = []
        for r in range(ngrp):
            r_t = const.tile(
                [128, NBLK * BD * gsizes[r]], FP8, tag=f"r{r}", name=f"r{r}",
            )
            r_ts.append(r_t)

        # --- input DMA: all rhs on the scalar HWDGE queue (the sync queue
        # trickles); small tensors on gpsimd SWDGE ---
        for b in range(NBLK):
            nc.scalar.dma_start(
                qlhsT_t[:, b * 2 * ROWS:(b + 1) * 2 * ROWS], qlhsT_d[b]
            )
        for i in range(len(cb) - 1):
            nc.scalar.dma_start(rhs_t[:, 2 * cb[i]:2 * cb[i + 1]], rhs_ds[i][:])
        nc.gpsimd.dma_start(idp_t[:], idp_d[:])
        nc.gpsimd.dma_start(sel_t[:], sel_d[:])
        nc.gpsimd.dma_start(qclsT_t[:], qclsT_d[:])
        nc.gpsimd.dma_start(dclsT_t[:], dclsT_d[:])

        # One full PSUM bank: rotating T accumulator [128, 2, 2, 128] fp32 --
        # phase-subcolumn x block x doc; subcolumns folded after the copy out
        t_tile = tpsum.tile([128, 2, NBLK, BD], F32, tag="T")
        out_ps = opsum.tile([QPC, BD], F32, tag="out_ps")

        rhs_pairs = rhs_t[:].rearrange("p (n o) -> p o n", o=2)

        def aug_mm(ps_slice, b, c0, cw):
            nc.tensor.matmul(
                ps_slice,
                qlhsT_t[:, b * 2 * ROWS:(b + 1) * 2 * ROWS].rearrange(
                    "p (o m) -> p o m", o=2),
                rhs_pairs[:, :, c0:c0 + cw],
                start=True, stop=True,
                perf_mode=mybir.MatmulPerfMode.DoubleRow,
            )

        # --- aug matmuls + relu decode; blocks interleaved per 2-bank tile
        # pair so both consume each DMA chunk as it lands ---
        idp_ap = idp_t[:].rearrange("p (o m) -> p o m", o=2)

        ralt = 0
        nbk0 = BD * gsizes[0] // 512              # region-0 banks per block
        ntile0 = (nbk0 + 1) // 2

        def t0_block(bsel, started):
            npp0 = gsizes[0] // 2
            base = r_ts[0][:].rearrange(
                "p (b pp d par) -> p pp par b d", b=NBLK, d=BD, par=2)
            for pp in range(npp0):
                si = pp % 2
                nc.tensor.matmul(
                    t_tile[:, si, bsel, :], idp_ap, base[:, pp, :, bsel, :],
                    start=(not started and pp == 0),
                    stop=(bsel == NBLK - 1 and pp == npp0 - 1),
                    perf_mode=mybir.MatmulPerfMode.DoubleRow,
                    skip_group_check=True,
                )

        for b in range(NBLK):
            for t in range(ntile0):
                ps = psum.tile([128, 2, 512], F32, tag="aug")
                nk = min(2, nbk0 - t * 2)
                for k in range(nk):
                    aug_mm(ps[:, k, :], b, (t * 2 + k) * 512, 512)
                off = (b * nbk0 + t * 2) * 512
                dst = r_ts[0][:, off:off + nk * 512]
                src = ps[:].rearrange("p a t -> p (a t)")[:, 0:nk * 512]
                if ralt % 2 == 0:
                    nc.scalar.activation(
                        dst, src, mybir.ActivationFunctionType.Relu)
                else:
                    nc.vector.tensor_scalar_max(dst, src, 0.0)
                ralt += 1
            for r in range(1, ngrp):
                gr = gsizes[r]
                ps = psum.tile([128, 2, 512], F32, tag="aug")
                pr = ps[:].rearrange("p a t -> p (a t)")[:, 0:BD * gr]
                for c0 in range(0, BD * gr, 512):
                    cw = min(512, BD * gr - c0)
                    aug_mm(pr[:, c0:c0 + cw], b, reg_off[r] + c0, cw)
                dst = r_ts[r][:, b * BD * gr:(b + 1) * BD * gr]
                if ralt % 2 == 0:
                    nc.scalar.activation(
                        dst, pr, mybir.ActivationFunctionType.Relu)
                else:
                    nc.vector.tensor_scalar_max(dst, pr, 0.0)
                ralt += 1
            if b == 0:
                # T matmuls for block 0 overlap block 1's aug sweep
                t0_block(0, started=False)

        # CLS matmuls (own PSUM bank; emitted after the sweep so they do not
        # head-of-line-block the aug stream while cls tensors stream in)
        for k in range(6):
            nc.tensor.matmul(
                out_ps[:],
                qclsT_t[:, k * QPC:(k + 1) * QPC],
                dclsT_t[:, k * BD:(k + 1) * BD],
                start=(k == 0),
                stop=False,
            )

        # --- T matmuls: block 1's region-0 phases (block 0 ran during the
        # sweep), then small regions; ppsub fold after the bf16 copy out ---
        tsb = []
        cp = work.tile([128, ngrp, 2 * NBLK * BD], BF16, tag="tsb_cp")
        t0_block(1, started=True)
        if ngrp % 2:
            nc.scalar.copy(cp[:, 0, :], t_tile[:])
        else:
            nc.vector.tensor_copy(cp[:, 0, :], t_tile[:])
        tsb.append(cp[:, 0, :])
        for r in range(1, ngrp):
            gr = gsizes[r]
            npp = gr // 2
            nq = (npp + 1) // 2
            for q in range(nq):
                pps = [2 * q] if 2 * q + 1 >= npp else [2 * q, 2 * q + 1]
                base = r_ts[r][:].rearrange(
                    "p (b pp d par) -> p pp par b d",
                    b=NBLK, d=BD, par=2,
                )
                for si, pp in enumerate(pps):
                    nc.tensor.matmul(
                        t_tile[:, si, :, :], idp_ap, base[:, pp],
                        start=(q == 0 and si == 0),
                        stop=(q == nq - 1 and si == len(pps) - 1),
                        perf_mode=mybir.MatmulPerfMode.DoubleRow,
                        skip_group_check=True,
                    )
            if r % 2:
                nc.scalar.copy(cp[:, r, :], t_tile[:])
            else:
                nc.vector.tensor_copy(cp[:, r, :], t_tile[:])
            tsb.append(cp[:, r, :])

        # fold ppsub pairs: [128, (ppsub, b*d)] -> [128, b*d]
        tf = work.tile([128, ngrp, NBLK * BD], BF16, tag="tsb_f")
        tsf = []
        for r in range(ngrp):
            v = tsb[r].rearrange("p (s n) -> p s n", s=2)
            nc.vector.tensor_tensor(
                tf[:, r, :], v[:, 0, :], v[:, 1, :], op=mybir.AluOpType.add)
            tsf.append(tf[:, r, :])
        tsb = tsf

        # --- group max (exact reproduction of the reference per-row max) ---
        tok_t = work.tile([128, NBLK * BD], BF16, tag="tok")
        if ngrp == 1:
            nc.vector.tensor_copy(tok_t[:], tsb[0])
        else:
            acc = tsb[0]
            for r in range(1, ngrp):
                nxt = tok_t if r == ngrp - 1 else work.tile(
                    [128, NBLK * BD], BF16, tag=f"tmax{r}", name=f"tmax{r}"
                )
                nc.vector.tensor_tensor(
                    nxt[:], tsb[r], acc, op=mybir.AluOpType.max
                )
                acc = nxt[:]
        # --- weighted token sum into out_ps ---
        for b in range(NBLK):
            nc.tensor.matmul(
                out_ps[:],
                sel_t[:, b * QPC:(b + 1) * QPC],
                tok_t[:, b * BD:(b + 1) * BD],
                start=False,
                stop=(b == NBLK - 1),
            )

        outsb = work.tile([QPC, BD], F32, tag="outsb")
        nc.scalar.copy(outsb[:], out_ps[:])
        nc.sync.dma_start(out_d[:], outsb[:])
        if debug_taps:
            nc.sync.dma_start(r0dbg_d[:], r_ts[0][:])
            nc.sync.dma_start(tokdbg_d[:], tok_t[:])
            tsbf = work.tile([128, ngrp * NBLK * BD], F32, tag="tsbf")
            for r in range(ngrp):
                nc.vector.tensor_copy(
                    tsbf[:, r * NBLK * BD:(r + 1) * NBLK * BD], tsb[r])
            nc.sync.dma_start(tdbg_d[:], tsbf[:])

    nc.compile()
    return nc


_NC_CACHE = {}


def _get_nc(gsizes, debug_taps=False):
    key = (gsizes, debug_taps)
    if key not in _NC_CACHE:
        _NC_CACHE[key] = build_nc(gsizes, debug_taps)
    return _NC_CACHE[key]


def _digit_onehot(ids, scale):
    ids = ids.astype(np.int64)
    oh = np.zeros(ids.shape + (KD,), np.float32)
    flat = oh.reshape(-1, KD)
    fid = ids.reshape(-1)
    idx = np.arange(fid.size)
    for t in range(NDIG):
        flat[idx, t * DIG + (fid // (DIG ** t)) % DIG] = scale
    return oh


def _doc_groups(did, qid):
    """Candidate filter + duplicate-rank grouping of doc positions.

    A doc position whose token id appears in no query token can never
    produce a full id match, so its relu-decoded contribution is exactly 0
    for every (query token, doc) pair -- pruning it is exact work elision.
    Survivors are then grouped by within-doc duplicate rank so ids are
    unique per doc within a group."""
    qset = np.zeros(int(max(did.max(), qid.max())) + 1, bool)
    qset[qid.reshape(-1)] = True
    keep = qset[did]                              # [BD, LD]
    ranks = np.full(did.shape, -1, dtype=np.int64)
    for d in range(BD):
        seen = {}
        for j in range(LD):
            if not keep[d, j]:
                continue
            v = int(did[d, j])
            r = seen.get(v, 0)
            seen[v] = r + 1
            ranks[d, j] = r
    nrank = max(1, int(ranks.max()) + 1)
    gsizes = []
    pos = []
    for r in range(nrank):
        cnt = (ranks == r).sum(axis=1)
        gr = _round_g(cnt.max()) if r == 0 else max(2, int(np.ceil(cnt.max() / 2)) * 2)
        p = np.full((BD, gr), -1, np.int64)
        for d in range(BD):
            js = np.nonzero(ranks[d] == r)[0]
            p[d, :len(js)] = js
        gsizes.append(gr)
        pos.append(p)
    return tuple(gsizes), pos


def _hilo8(x):
    hi = x.astype(E4)
    lo = (x - hi.astype(np.float32)).astype(E4)
    return hi.astype(np.float32), lo.astype(np.float32)


def make_in_maps(qte, dte, qce, dce, qid, did, qam):
    # SEP mask + CLS drop -> per-token weights
    sep = qam.sum(1) - 1
    qm = qam.astype(np.float32).copy()
    qm[np.arange(BQ), sep] = 0.0
    w = qm.copy()
    w[:, 0] = 0.0

    # per-core candidate filter: prune doc positions against each core's
    # own 8 queries; common (max) group sizes so all cores run one program
    percore = [_doc_groups(did, qid[c * QPC:(c + 1) * QPC]) for c in range(NCORES)]
    ngrp = max(len(g) for g, _ in percore)
    glist = []
    for r in range(ngrp):
        mx = max((g[r] if r < len(g) else 0) for g, _ in percore)
        glist.append(_round_g(mx) if r == 0 else max(2, int(np.ceil(mx / 2)) * 2))
    gsizes = tuple(glist)
    ndp = BD * sum(gsizes)
    cb = _chunks(ndp)

    # doc-side feature pairs [KP, ndp, 2] fp8
    doh = _digit_onehot(did, 1.0)                  # [BD, LD, KD]
    dh, dl = _hilo8(dte)                           # [BD, LD, TOK_D]
    dfeat = np.zeros((KP, BD * LD, 2), np.float32)
    dhf = dh.transpose(2, 0, 1).reshape(TOK_D, BD * LD)
    dlf = dl.transpose(2, 0, 1).reshape(TOK_D, BD * LD)
    dfeat[0:TOK_D, :, 0] = dhf
    dfeat[0:TOK_D, :, 1] = dhf
    dfeat[TOK_D:2 * TOK_D, :, 0] = dlf
    dohf = doh.transpose(2, 0, 1).reshape(KD, BD * LD)
    for j in range(KD // 2):
        dfeat[2 * TOK_D + j, :, 0] = dohf[2 * j]
        dfeat[2 * TOK_D + j, :, 1] = dohf[2 * j + 1]
    dfeat[KP - 1, :, :] = 2.0

    def core_rhs(core):
        gsz_c, pos_c = percore[core]
        rhs = np.zeros((KP, ndp, 2), E4)
        off = 0
        for r, gr in enumerate(gsizes):
            idx = np.full((BD, gr), -1, np.int64)
            if r < len(gsz_c):
                idx[:, :gsz_c[r]] = pos_c[r]
            # column order (pp, d, par): slot = 2pp+par
            iv = np.ascontiguousarray(
                idx.reshape(BD, gr // 2, 2).transpose(1, 0, 2))
            dv = np.broadcast_to(np.arange(BD).reshape(1, BD, 1), iv.shape)
            flat_i = iv.reshape(-1)
            flat_d = dv.reshape(-1)
            src = np.where(flat_i >= 0, flat_d * LD + np.maximum(flat_i, 0), 0)
            block = dfeat[:, src, :].astype(E4)
            block[:, flat_i < 0, :] = 0
            rhs[:, off:off + BD * gr] = block
            off += BD * gr
        return rhs

    qoh = _digit_onehot(qid, C)                    # [BQ, LQ, KD]
    dclsT = np.ascontiguousarray(
        dce.T.reshape(CLS_D // 128, 128, BD).transpose(1, 0, 2)
        .reshape(128, 6 * BD)).astype(ml_dtypes.bfloat16)
    idp = np.zeros((128, 2 * 128), dtype=E4)
    for p in range(128):
        idp[p, p] = 1.0
        idp[p, 128 + p] = 1.0

    in_maps = []
    for c in range(NCORES):
        rhs = core_rhs(c)
        rhs_chunks = {
            f"rhs{i}": np.ascontiguousarray(
                rhs[:, cb[i]:cb[i + 1], :].reshape(KP, -1))
            for i in range(len(cb) - 1)
        }
        qs = slice(c * QPC, (c + 1) * QPC)
        qte_c, qoh_c, w_c = qte[qs], qoh[qs], w[qs]

        qlhsT = np.zeros((NBLK, KP, 2, ROWS), np.float32)
        for b in range(NBLK):
            blk = qte_c[b * 4:(b + 1) * 4].reshape(ROWS, TOK_D)
            qh, ql = _hilo8(blk)
            qlhsT[b, 0:TOK_D, 0] = qh.T
            qlhsT[b, 0:TOK_D, 1] = ql.T
            qlhsT[b, TOK_D:2 * TOK_D, 0] = qh.T
            ohb = qoh_c[b * 4:(b + 1) * 4].reshape(ROWS, KD).T
            for j in range(KD // 2):
                qlhsT[b, 2 * TOK_D + j, 0] = ohb[2 * j]
                qlhsT[b, 2 * TOK_D + j, 1] = ohb[2 * j + 1]
            qlhsT[b, KP - 1, :, :] = BIAS

        sel = np.zeros((ROWS, NBLK, QPC), np.float32)
        for b in range(NBLK):
            for qq in range(4):
                ql_ = b * 4 + qq
                sel[qq * 32:(qq + 1) * 32, b, ql_] = w_c[ql_]
        sel = sel.reshape(ROWS, NBLK * QPC)

        qclsT = np.ascontiguousarray(
            qce[qs].T.reshape(CLS_D // 128, 128, QPC).transpose(1, 0, 2)
            .reshape(128, 6 * QPC)).astype(ml_dtypes.bfloat16)

        im = {
            "qlhsT": qlhsT.reshape(NBLK, KP, 2 * ROWS).astype(E4),
            "sel": sel.astype(ml_dtypes.bfloat16),
            "qclsT": qclsT,
            "dclsT": dclsT,
            "idp": idp,
        }
        im.update(rhs_chunks)
        in_maps.append(im)
    return gsizes, in_maps


def run(gsizes, in_maps, trace=False, debug_taps=False, **kwargs):
    nc = _get_nc(gsizes, debug_taps)
    return run_bass_kernel_spmd(
        nc, in_maps, core_ids=list(range(NCORES)), trace=trace, **kwargs
    )


def kernel(
    query_tok_embs,
    doc_tok_embs,
    query_cls_emb,
    doc_cls_emb,
    query_input_ids,
    doc_input_ids,
    query_attention_mask,
):
    qte = np.ascontiguousarray(np.asarray(query_tok_embs, np.float32))
    dte = np.ascontiguousarray(np.asarray(doc_tok_embs, np.float32))
    qce = np.ascontiguousarray(np.asarray(query_cls_emb, np.float32))
    dce = np.ascontiguousarray(np.asarray(doc_cls_emb, np.float32))
    qid = np.asarray(query_input_ids).astype(np.int64)
    did = np.asarray(doc_input_ids).astype(np.int64)
    qam = np.asarray(query_attention_mask).astype(np.int64)

    gsizes, in_maps = make_in_maps(qte, dte, qce, dce, qid, did, qam)
    res = run(gsizes, in_maps)
    out = np.concatenate([r["out"] for r in res.results], axis=0)
    return np.ascontiguousarray(out.astype(np.float32))
